# revision 15
# baseline (speedup 1.0000x reference)
"""Trainium2 Bass kernel for nn_CellLayer_25752623907073.

The reference is an init-guess network (MLP/S4D stack) followed by a DEER
quasi-Newton parallel solve of a GRU recurrence.  DEER is a contraction: it
converges to the sequential GRU trajectory from any initial guess, so the
init-guess network has no effect on the output and the task reduces to
evaluating the GRU trajectory.

The kernel solves the GRU by quasi-DEER fixed-point iteration with a
DIAGONAL linear solve: each round evaluates all gates in parallel at the
lagged previous iterate h~[t-1], then propagates the exact diagonal
recurrence h[t] = z[t]*h[t-1] + (1-z[t])*a[t] along the sequence with DVE
tensor_tensor_scan ops (state fp32 inside the scan).  The fixed point is
the true trajectory; measured contraction ~0.26/round, so ROUNDS=4 puts
iteration error under the bf16 floor (total rel err ~8e-3 vs 2e-2 gate).

Implementation notes:
- State is carried as H' = (h+1)/2, which turns the a-gate tanh into a
  sigmoid (tanh(x) = 2*sigmoid(2x)-1) with all affine corrections folded
  into host-side weight/bias transforms:
      H'[t] = z[t]*H'[t-1] - (z[t]-1)*s[t],   s = sigmoid(2*aarg).
  Only the sigmoid ACT table is ever needed (one 1283ns table load, warmed
  during the input DMA), and no extra DVE op for the *2-1 affine.
- Sharding: 8 cores = 4 batches x 2 halves, no collectives.  Each core owns
  MARG+1024 positions; the window splits into two 528-col segments stacked
  on partitions (seg0 -> 0:64, seg1 -> 64:128) so every ACT/DVE op covers
  2x columns; matmuls use block-diagonal stationaries.  Segment scan
  initials are H'=0.5 (h=0) every round; the injected boundary error
  decays below 1e-3 within the MARG warm-in cols, which are discarded.
- The first-half core's seg0 warm-in cols have no real inputs: a dedicated
  x2 mask row drives the z pre-activation to +32 there, so z=1 and the
  scan passes the initial state through exactly (no mask op needed).
- Per 264-col tile and round: 7 bf16 matmuls (x-parts emitted early /
  h-parts close each psum so sigmoids fire right after the scan lands),
  sigmoids for r/z/a, t1=(ha+bn'')*r on DVE, a PE identity-matmul folding
  t1 onto the ia psum bank, then g=(z-1)*s and the scan back-to-back on
  DVE.  Per-engine emission follows steady-state readiness: the
  completion-counter semaphores entangle waiters when instructions park.
- Round 0 runs the generic code against an H'=0.5 buffer (h=0 exactly).
- y is DMA'd as bf16 H' (the last round's tile-1 scan is chunked so the
  DMAs pipeline on the SP/ACT HWDGE queues); the host maps 2*H'-1.
"""

import numpy as np
import ml_dtypes

import concourse.bacc as bacc
import concourse.mybir as mybir
import concourse.tile as tile
from concourse.bass_utils import run_bass_kernel_spmd

F32 = mybir.dt.float32
BF16 = mybir.dt.bfloat16
FP16 = mybir.dt.float16
AF = mybir.ActivationFunctionType
ALU = mybir.AluOpType

B, L, NIN, H = 4, 2048, 32, 64
TPC = L // 2          # timesteps per core
MARG = 16             # warm-in columns per segment (discarded)
SL = MARG + 512       # segment length (528)
NW = MARG + TPC       # window length (1040)
NSEG = 2
TLS = [276, 252]      # tile column sizes (sum = SL)
FIN_CHUNK = 252       # last-round scan/DMA chunk size
ROUNDS = 4
N_CORES = 8
ZBIG = 32.0           # z pre-act saturation for masked warm-in cols
XR = 2 * (NIN + 1) + 1  # x2 rows: 2 segs x (x + ones) + seg0 mask row

# blob columns (bf16, [128, BLOBCOLS]) — DMA'd in 4 pieces:
#   A (SP):  [0:513]  wxr|wxz|wxa block-diag stationaries (x rows, bias row,
#            mask row), I128, bn'' col
#   B (DVE): x2 tile-0 cols
#   C (SP):  x2 tile-1 cols
#   D (SP):  uhr|uhz|uha block-diag stationaries (needed from round 0)
W0OFF = {"wxr": 0, "wxz": 128, "wxa": 256}     # round-0 bias variant
BNOFF = 384            # col 384 = bn'' (rounds>0), col 385 = 2*bn (round 0)
XOFF = 386             # DMA-A covers partitions 0:XR, cols [0:XOFF+SL)
IDOFF = XOFF + SL      # DMA-B: ident, generic wx, uh stationaries
WOFF = {"wxr": IDOFF + 128, "wxz": IDOFF + 256, "wxa": IDOFF + 384}
UOFF = {"uhr": IDOFF + 512, "uhz": IDOFF + 640, "uha": IDOFF + 768}
BLOBCOLS = IDOFF + 896


def _build_program():
    nc = bacc.Bacc("TRN2", debug=False)

    wx = nc.declare_dram_parameter("wx", [128, BLOBCOLS], BF16, isOutput=False)
    yout = nc.declare_dram_parameter("y", [128, 512], FP16, isOutput=True)

    with tile.TileContext(nc) as tc:
        with (
            tc.tile_pool(name="const", bufs=1) as cpool,
            tc.tile_pool(name="tmp", bufs=2) as tmp,
            tc.tile_pool(name="psum_r", bufs=2, space="PSUM") as psum_r,
            tc.tile_pool(name="psum_z", bufs=2, space="PSUM") as psum_z,
            tc.tile_pool(name="psum_a1", bufs=2, space="PSUM") as psum_a1,
            tc.tile_pool(name="psum_ia", bufs=2, space="PSUM") as psum_ia,
        ):
            t_wx = cpool.tile([128, BLOBCOLS], BF16)
            nc.sync.dma_start(t_wx[0:XR, 0:XOFF + TLS[0]],
                              wx[0:XR, 0:XOFF + TLS[0]])
            nc.sync.dma_start(t_wx[0:XR, XOFF + TLS[0]:XOFF + SL],
                              wx[0:XR, XOFF + TLS[0]:XOFF + SL])
            nc.sync.dma_start(t_wx[:, IDOFF:BLOBCOLS],
                              wx[:, IDOFF:BLOBCOLS])

            wst = {k: t_wx[:, off:off + 128] for k, off in UOFF.items()}
            wst["ident"] = t_wx[:, IDOFF:IDOFF + 128]
            wx_x = {k: t_wx[0:XR, WOFF[k]:WOFF[k] + 128]
                    for k in ("wxr", "wxz", "wxa")}
            wx_x0 = {k: t_wx[0:XR, W0OFF[k]:W0OFF[k] + 128]
                     for k in ("wxr", "wxz", "wxa")}

            # PE p-state warm-up: dummy matmuls on a zeroed tile so the ramp
            # to full clock runs during the input DMA.
            t_zero = cpool.tile([128, 264], BF16)
            nc.vector.memset(t_zero[:], 0.0)
            for i in range(2):
                p_w = psum_ia.tile([128, 264], F32, tag="pia", name=f"warm{i}")
                nc.tensor.matmul(p_w[:], t_zero[:, 0:128], t_zero[:],
                                 start=True, stop=True)

            # warm the sigmoid ACT table during the input DMA
            t_warm = cpool.tile([1, 1], F32)
            nc.vector.memset(t_warm[:], 0.0)
            nc.scalar.activation(t_warm[:], t_warm[:], AF.Sigmoid)

            # greedy-scheduler engine reservations: small always-ready ops
            # emitted just before a critical op whose deps are in ack-flight,
            # so the list scheduler doesn't give the slot to a long off-path
            # op instead.
            f_dve = cpool.tile([128, 180], FP16)
            f_act = cpool.tile([128, 170], F32)

            # bn'' / 2bn as f32 via DVE so t1 never waits on the DMA sem;
            # the blob carries rows 0:64 only (partition-packed first DMA),
            # ACT replicates into partitions 64:128.
            t_bn = cpool.tile([128, 2], F32)
            nc.vector.tensor_copy(t_bn[0:H, :], t_wx[0:H, BNOFF:BNOFF + 2])
            nc.scalar.activation(t_bn[H:128, :], t_bn[0:H, :], AF.Copy)

            # H' double buffers: col 0 = 0.5 (h=0 before the segment), col
            # 1+c = state at segment-local col c.  hbuf[0] is fully 0.5 so
            # round 0 is the generic code evaluated at h=0.
            hbuf = [cpool.tile([128, 1 + SL], FP16, name=f"h{i}")
                    for i in range(2)]
            nc.vector.memset(hbuf[0][:, 0:1], 0.5)
            nc.vector.memset(hbuf[1][:, 0:1], 0.5)

            def x2(c0, tl):
                return t_wx[0:XR, XOFF + c0:XOFF + c0 + tl]

            def emit_round(k):
                hprev = hbuf[k % 2]
                hnew = hbuf[(k + 1) % 2]
                ps = {}
                gate = {}
                # PE: x-part starts (ready as soon as the psum bank WAR
                # clears, run in earlier idle slots), then h-part closes in
                # readiness order, then the ident folds.
                wxk = wx_x0 if k == 0 else wx_x
                for t, tl in enumerate(TLS):
                    c0 = t * TLS[0]
                    xa = x2(c0, tl)
                    p_r = psum_r.tile([128, tl], F32, tag="pr",
                                      name=f"pr_{k}_{t}")
                    p_z = psum_z.tile([128, tl], F32, tag="pz",
                                      name=f"pz_{k}_{t}")
                    p_a1 = psum_a1.tile([128, tl], F32, tag="pa1",
                                        name=f"pa1_{k}_{t}")
                    p_ia = psum_ia.tile([128, tl], F32, tag="pia",
                                        name=f"pia_{k}_{t}")
                    last = k == 0
                    nc.tensor.matmul(p_r[:], wxk["wxr"], xa,
                                     start=True, stop=last)
                    nc.tensor.matmul(p_z[:], wxk["wxz"], xa,
                                     start=True, stop=last,
                                     skip_group_check=True)
                    nc.tensor.matmul(p_ia[:], wxk["wxa"], xa,
                                     start=True, stop=False,
                                     skip_group_check=True)
                    ps[t] = (p_r, p_z, p_a1, p_ia)
                if k > 0:
                    for t, tl in enumerate(TLS):
                        c0 = t * TLS[0]
                        hp = hprev[:, c0:c0 + tl]
                        p_r, p_z, p_a1, p_ia = ps[t]
                        nc.tensor.matmul(p_r[:], wst["uhr"], hp,
                                         start=False, stop=True,
                                         skip_group_check=True)
                        nc.tensor.matmul(p_a1[:], wst["uha"], hp,
                                         start=True, stop=True,
                                         skip_group_check=True)
                        nc.tensor.matmul(p_z[:], wst["uhz"], hp,
                                         start=False, stop=True,
                                         skip_group_check=True)

                gbuf = tmp.tile([128, SL], FP16, tag="gt", name=f"gt{k}")
                for t, tl in enumerate(TLS):
                    c0 = t * TLS[0]
                    rt = tmp.tile([128, tl], FP16, tag="rt", name=f"rt{k}_{t}")
                    zt = tmp.tile([128, tl], FP16, tag="zt", name=f"zt{k}_{t}")
                    t1 = tmp.tile([128, tl], FP16, tag="t1", name=f"t1{k}_{t}")
                    at = tmp.tile([128, tl], FP16, tag="at", name=f"at{k}_{t}")
                    gt = gbuf[:, c0:c0 + tl]
                    gate[t] = (rt, zt, t1, at, gt)

                def emit_t1(t):
                    rt, zt, t1, at, gt = gate[t]
                    p_r, p_z, p_a1, p_ia = ps[t]
                    if k == 0:
                        nc.vector.tensor_scalar_mul(t1[:], rt[:],
                                                    t_bn[:, 1:2])
                    else:
                        nc.vector.scalar_tensor_tensor(
                            t1[:], in0=p_a1[:], scalar=t_bn[:, 0:1],
                            in1=rt[:], op0=ALU.add, op1=ALU.mult)
                    nc.tensor.matmul(p_ia[:], wst["ident"], t1[:],
                                     start=False, stop=True,
                                     skip_group_check=True)

                # ACT/DVE in readiness order
                nc.scalar.activation(gate[0][0][:], ps[0][0][:], AF.Sigmoid)
                nc.scalar.activation(gate[0][1][:], ps[0][1][:], AF.Sigmoid)
                emit_t1(0)
                nc.scalar.activation(gate[1][0][:], ps[1][0][:], AF.Sigmoid)
                nc.scalar.activation(f_act[:], gate[0][1][:, 0:170], AF.Sigmoid)
                nc.scalar.activation(gate[0][3][:], ps[0][3][:], AF.Sigmoid)
                emit_t1(1)
                nc.scalar.activation(gate[1][1][:], ps[1][1][:], AF.Sigmoid)

                dmaq = [nc.sync, nc.scalar]
                dmai = [0]

                def emit_dma(blo, bhi):
                    # one 2D DMA moves both segments: yout rows 0:64 = seg0,
                    # rows 64:128 = seg1 (host splits)
                    ylo, yhi = blo - 1 - MARG, bhi - 1 - MARG
                    q0 = dmaq[dmai[0] % 2]
                    dmai[0] += 1
                    q0.dma_start(yout[:, ylo:yhi], hnew[:, blo:bhi])

                rt0, zt0, t10, at0, gt0 = gate[0]
                nc.vector.scalar_tensor_tensor(
                    gt0[:], in0=zt0[:], scalar=1.0, in1=at0[:],
                    op0=ALU.subtract, op1=ALU.mult)
                nc.vector.tensor_copy(f_dve[:], gate[0][3][:, 0:180])
                nc.vector.tensor_tensor_scan(
                    hnew[:, 1:1 + TLS[0]], zt0[:], gt0[:], 0.5,
                    ALU.mult, ALU.subtract)
                if k == ROUNDS - 1:
                    emit_dma(1 + MARG, 1 + TLS[0])

                nc.scalar.activation(gate[1][3][:], ps[1][3][:], AF.Sigmoid)
                rt1, zt1, t11, at1, gt1 = gate[1]
                nc.vector.scalar_tensor_tensor(
                    gt1[:], in0=zt1[:], scalar=1.0, in1=at1[:],
                    op0=ALU.subtract, op1=ALU.mult)
                if k < ROUNDS - 1:
                    nc.vector.tensor_tensor_scan(
                        hnew[:, 1 + TLS[0]:1 + SL], zt1[:], gt1[:],
                        hnew[:, TLS[0]:TLS[0] + 1], ALU.mult, ALU.subtract)
                else:
                    c0 = TLS[0]
                    while c0 < SL:
                        ce = min(c0 + FIN_CHUNK, SL)
                        nc.vector.tensor_tensor_scan(
                            hnew[:, 1 + c0:1 + ce],
                            zt1[:, c0 - TLS[0]:ce - TLS[0]],
                            gt1[:, c0 - TLS[0]:ce - TLS[0]],
                            hnew[:, c0:c0 + 1], ALU.mult, ALU.subtract)
                        emit_dma(1 + c0, 1 + ce)
                        c0 = ce

            for k in range(ROUNDS):
                emit_round(k)

    nc.compile()
    return nc


_CACHE = {}


def kernel(**inputs):
    xs = np.asarray(inputs["xs"], np.float32)
    w_ih = np.asarray(inputs["w_ih"], np.float32)
    w_hh = np.asarray(inputs["w_hh"], np.float32)
    b_gru = np.asarray(inputs["b_gru"], np.float32)
    bn_gru = np.asarray(inputs["bn_gru"], np.float32)

    if "nc" not in _CACHE:
        _CACHE["nc"] = _build_program()
    nc = _CACHE["nc"]

    # H' = (h+1)/2 substitution + tanh->sigmoid doubling, folded into the
    # stationaries:  gpre = Wg x + (bg - Ug@1) + (2 Ug) H'   for g in r,z
    #   2*aarg = (2 Wa) x + 2 ba + r*( (4 Ua) H' + 2(bn - Ua@1) )
    base = np.zeros((128, BLOBCOLS), np.float32)
    gmul = {"wxr": 1.0, "wxz": 1.0, "wxa": 2.0}
    umul = {"uhr": 2.0, "uhz": 2.0, "uha": 4.0}
    for gi, key in enumerate(("wxr", "wxz", "wxa")):
        wg = w_ih[gi * H:(gi + 1) * H]          # (H, NIN)
        ug = w_hh[gi * H:(gi + 1) * H]          # (H, H)
        if key == "wxa":
            bg = 2.0 * b_gru[gi * H:(gi + 1) * H]
            bg0 = bg
        else:
            bg = b_gru[gi * H:(gi + 1) * H] - ug.sum(axis=1)
            bg0 = b_gru[gi * H:(gi + 1) * H]
        for s in range(NSEG):
            r0 = s * (NIN + 1)
            for off, bias in ((WOFF[key], bg), (W0OFF[key], bg0)):
                base[r0:r0 + NIN, off + s * H:off + s * H + H] = \
                    gmul[key] * wg.T
                base[r0 + NIN, off + s * H:off + s * H + H] = bias
    # seg0 mask row: saturate z -> 1 on masked cols (state pass-through)
    base[XR - 1, WOFF["wxz"]:WOFF["wxz"] + H] = ZBIG
    base[XR - 1, W0OFF["wxz"]:W0OFF["wxz"] + H] = ZBIG
    for gi, key in enumerate(("uhr", "uhz", "uha")):
        ug = w_hh[gi * H:(gi + 1) * H]
        for s in range(NSEG):
            base[s * H:(s + 1) * H,
                 UOFF[key] + s * H:UOFF[key] + (s + 1) * H] = umul[key] * ug.T
    base[:, IDOFF:IDOFF + 128] = np.eye(128, dtype=np.float32)
    bnpp = 2.0 * (bn_gru - w_hh[2 * H:].sum(axis=1))
    base[0:H, BNOFF] = bnpp
    base[0:H, BNOFF + 1] = 2.0 * bn_gru

    in_maps = []
    for core in range(N_CORES):
        bi, half = core // 2, core % 2
        p0 = half * TPC - MARG
        xw = np.zeros((NW, NIN), np.float32)
        lo = max(0, p0)
        xw[lo - p0:] = xs[bi, lo:p0 + NW]
        blob = base.copy()
        for s in range(NSEG):
            r0 = s * (NIN + 1)
            xsg = xw[512 * s:512 * s + SL]       # (SL, NIN)
            blob[r0:r0 + NIN, XOFF:XOFF + SL] = xsg.T
            blob[r0 + NIN, XOFF:XOFF + SL] = 1.0
        if half == 0:
            blob[XR - 1, XOFF:XOFF + MARG] = 1.0
        in_maps.append({"wx": blob.astype(ml_dtypes.bfloat16)})

    results = run_bass_kernel_spmd(nc, in_maps, list(range(N_CORES))).results

    out = np.empty((B, L, H), np.float32)
    for core in range(N_CORES):
        bi, half = core // 2, core % 2
        y = np.asarray(results[core]["y"]).astype(np.float32)   # (128, 512)
        yh = np.concatenate([y[0:H], y[H:128]], axis=1)          # (64, 1024)
        out[bi, half * TPC:(half + 1) * TPC] = 2.0 * yh.T - 1.0
    return out


# revision 16
# speedup vs baseline: 1.0014x; 1.0014x over previous
"""Trainium2 Bass kernel for nn_CellLayer_25752623907073.

The reference is an init-guess network (MLP/S4D stack) followed by a DEER
quasi-Newton parallel solve of a GRU recurrence.  DEER is a contraction: it
converges to the sequential GRU trajectory from any initial guess, so the
init-guess network has no effect on the output and the task reduces to
evaluating the GRU trajectory.

The kernel solves the GRU by quasi-DEER fixed-point iteration with a
DIAGONAL linear solve: each round evaluates all gates in parallel at the
lagged previous iterate h~[t-1], then propagates the exact diagonal
recurrence h[t] = z[t]*h[t-1] + (1-z[t])*a[t] along the sequence with DVE
tensor_tensor_scan ops (state fp32 inside the scan).  The fixed point is
the true trajectory; measured contraction ~0.26/round, so ROUNDS=4 puts
iteration error under the bf16 floor (total rel err ~8e-3 vs 2e-2 gate).

Implementation notes:
- State is carried as H' = (h+1)/2, which turns the a-gate tanh into a
  sigmoid (tanh(x) = 2*sigmoid(2x)-1) with all affine corrections folded
  into host-side weight/bias transforms:
      H'[t] = z[t]*H'[t-1] - (z[t]-1)*s[t],   s = sigmoid(2*aarg).
  Only the sigmoid ACT table is ever needed (one 1283ns table load, warmed
  during the input DMA), and no extra DVE op for the *2-1 affine.
- Sharding: 8 cores = 4 batches x 2 halves, no collectives.  Each core owns
  MARG+1024 positions; the window splits into two 528-col segments stacked
  on partitions (seg0 -> 0:64, seg1 -> 64:128) so every ACT/DVE op covers
  2x columns; matmuls use block-diagonal stationaries.  Segment scan
  initials are H'=0.5 (h=0) every round; the injected boundary error
  decays below 1e-3 within the MARG warm-in cols, which are discarded.
- The first-half core's seg0 warm-in cols have no real inputs: a dedicated
  x2 mask row drives the z pre-activation to +32 there, so z=1 and the
  scan passes the initial state through exactly (no mask op needed).
- Per 264-col tile and round: 7 bf16 matmuls (x-parts emitted early /
  h-parts close each psum so sigmoids fire right after the scan lands),
  sigmoids for r/z/a, t1=(ha+bn'')*r on DVE, a PE identity-matmul folding
  t1 onto the ia psum bank, then g=(z-1)*s and the scan back-to-back on
  DVE.  Per-engine emission follows steady-state readiness: the
  completion-counter semaphores entangle waiters when instructions park.
- Round 0 runs the generic code against an H'=0.5 buffer (h=0 exactly).
- y is DMA'd as bf16 H' (the last round's tile-1 scan is chunked so the
  DMAs pipeline on the SP/ACT HWDGE queues); the host maps 2*H'-1.
"""

import numpy as np
import ml_dtypes

import concourse.bacc as bacc
import concourse.mybir as mybir
import concourse.tile as tile
from concourse.bass_utils import run_bass_kernel_spmd

F32 = mybir.dt.float32
BF16 = mybir.dt.bfloat16
FP16 = mybir.dt.float16
AF = mybir.ActivationFunctionType
ALU = mybir.AluOpType

B, L, NIN, H = 4, 2048, 32, 64
TPC = L // 2          # timesteps per core
MARG = 16             # warm-in columns per segment (discarded)
SL = MARG + 512       # segment length (528)
NW = MARG + TPC       # window length (1040)
NSEG = 2
TLS = [276, 252]      # tile column sizes (sum = SL)
FIN_CHUNK = 252       # last-round scan/DMA chunk size
ROUNDS = 4
N_CORES = 8
ZBIG = 32.0           # z pre-act saturation for masked warm-in cols
XR = 2 * (NIN + 1) + 1  # x2 rows: 2 segs x (x + ones) + seg0 mask row

# blob columns (bf16, [128, BLOBCOLS]) — DMA'd in 4 pieces:
#   A (SP):  [0:513]  wxr|wxz|wxa block-diag stationaries (x rows, bias row,
#            mask row), I128, bn'' col
#   B (DVE): x2 tile-0 cols
#   C (SP):  x2 tile-1 cols
#   D (SP):  uhr|uhz|uha block-diag stationaries (needed from round 0)
W0OFF = {"wxr": 0, "wxz": 128, "wxa": 256}     # round-0 bias variant
BNOFF = 384            # col 384 = bn'' (rounds>0), col 385 = 2*bn (round 0)
XOFF = 386             # DMA-A covers partitions 0:XR, cols [0:XOFF+SL)
IDOFF = XOFF + SL      # DMA-B: ident, generic wx, uh stationaries
WOFF = {"wxr": IDOFF + 128, "wxz": IDOFF + 256, "wxa": IDOFF + 384}
UOFF = {"uhr": IDOFF + 512, "uhz": IDOFF + 640, "uha": IDOFF + 768}
BLOBCOLS = IDOFF + 896


def _build_program():
    nc = bacc.Bacc("TRN2", debug=False)

    wx = nc.declare_dram_parameter("wx", [128, BLOBCOLS], BF16, isOutput=False)
    yout = nc.declare_dram_parameter("y", [128, 512], FP16, isOutput=True)

    with tile.TileContext(nc) as tc:
        with (
            tc.tile_pool(name="const", bufs=1) as cpool,
            tc.tile_pool(name="tmp", bufs=2) as tmp,
            tc.tile_pool(name="psum_r", bufs=2, space="PSUM") as psum_r,
            tc.tile_pool(name="psum_z", bufs=2, space="PSUM") as psum_z,
            tc.tile_pool(name="psum_a1", bufs=2, space="PSUM") as psum_a1,
            tc.tile_pool(name="psum_ia", bufs=2, space="PSUM") as psum_ia,
        ):
            t_wx = cpool.tile([128, BLOBCOLS], BF16)
            nc.sync.dma_start(t_wx[0:XR, 0:XOFF + TLS[0]],
                              wx[0:XR, 0:XOFF + TLS[0]])
            nc.sync.dma_start(t_wx[:, XOFF + TLS[0]:IDOFF + 128],
                              wx[:, XOFF + TLS[0]:IDOFF + 128])
            nc.sync.dma_start(t_wx[:, IDOFF + 128:BLOBCOLS],
                              wx[:, IDOFF + 128:BLOBCOLS])

            wst = {k: t_wx[:, off:off + 128] for k, off in UOFF.items()}
            wst["ident"] = t_wx[:, IDOFF:IDOFF + 128]
            wx_x = {k: t_wx[0:XR, WOFF[k]:WOFF[k] + 128]
                    for k in ("wxr", "wxz", "wxa")}
            wx_x0 = {k: t_wx[0:XR, W0OFF[k]:W0OFF[k] + 128]
                     for k in ("wxr", "wxz", "wxa")}

            # PE p-state warm-up: dummy matmuls on a zeroed tile so the ramp
            # to full clock runs during the input DMA.
            t_zero = cpool.tile([128, 264], BF16)
            nc.vector.memset(t_zero[:], 0.0)
            for i in range(2):
                p_w = psum_ia.tile([128, 264], F32, tag="pia", name=f"warm{i}")
                nc.tensor.matmul(p_w[:], t_zero[:, 0:128], t_zero[:],
                                 start=True, stop=True)

            # warm the sigmoid ACT table during the input DMA
            t_warm = cpool.tile([1, 1], F32)
            nc.vector.memset(t_warm[:], 0.0)
            nc.scalar.activation(t_warm[:], t_warm[:], AF.Sigmoid)

            # greedy-scheduler engine reservations: small always-ready ops
            # emitted just before a critical op whose deps are in ack-flight,
            # so the list scheduler doesn't give the slot to a long off-path
            # op instead.
            f_dve = cpool.tile([128, 180], FP16)
            f_act = cpool.tile([128, 170], F32)

            # bn'' / 2bn as f32 via DVE so t1 never waits on the DMA sem;
            # the blob carries rows 0:64 only (partition-packed first DMA),
            # ACT replicates into partitions 64:128.
            t_bn = cpool.tile([128, 2], F32)
            nc.vector.tensor_copy(t_bn[0:H, :], t_wx[0:H, BNOFF:BNOFF + 2])
            nc.scalar.activation(t_bn[H:128, :], t_bn[0:H, :], AF.Copy)

            # H' double buffers: col 0 = 0.5 (h=0 before the segment), col
            # 1+c = state at segment-local col c.  hbuf[0] is fully 0.5 so
            # round 0 is the generic code evaluated at h=0.
            hbuf = [cpool.tile([128, 1 + SL], FP16, name=f"h{i}")
                    for i in range(2)]
            nc.vector.memset(hbuf[0][:, 0:1], 0.5)
            nc.vector.memset(hbuf[1][:, 0:1], 0.5)

            def x2(c0, tl):
                return t_wx[0:XR, XOFF + c0:XOFF + c0 + tl]

            def emit_round(k):
                hprev = hbuf[k % 2]
                hnew = hbuf[(k + 1) % 2]
                ps = {}
                gate = {}
                # PE: x-part starts (ready as soon as the psum bank WAR
                # clears, run in earlier idle slots), then h-part closes in
                # readiness order, then the ident folds.
                wxk = wx_x0 if k == 0 else wx_x
                for t, tl in enumerate(TLS):
                    c0 = t * TLS[0]
                    xa = x2(c0, tl)
                    p_r = psum_r.tile([128, tl], F32, tag="pr",
                                      name=f"pr_{k}_{t}")
                    p_z = psum_z.tile([128, tl], F32, tag="pz",
                                      name=f"pz_{k}_{t}")
                    p_a1 = psum_a1.tile([128, tl], F32, tag="pa1",
                                        name=f"pa1_{k}_{t}")
                    p_ia = psum_ia.tile([128, tl], F32, tag="pia",
                                        name=f"pia_{k}_{t}")
                    last = k == 0
                    nc.tensor.matmul(p_r[:], wxk["wxr"], xa,
                                     start=True, stop=last)
                    nc.tensor.matmul(p_z[:], wxk["wxz"], xa,
                                     start=True, stop=last,
                                     skip_group_check=True)
                    nc.tensor.matmul(p_ia[:], wxk["wxa"], xa,
                                     start=True, stop=False,
                                     skip_group_check=True)
                    ps[t] = (p_r, p_z, p_a1, p_ia)
                if k > 0:
                    for t, tl in enumerate(TLS):
                        c0 = t * TLS[0]
                        hp = hprev[:, c0:c0 + tl]
                        p_r, p_z, p_a1, p_ia = ps[t]
                        nc.tensor.matmul(p_r[:], wst["uhr"], hp,
                                         start=False, stop=True,
                                         skip_group_check=True)
                        nc.tensor.matmul(p_a1[:], wst["uha"], hp,
                                         start=True, stop=True,
                                         skip_group_check=True)
                        nc.tensor.matmul(p_z[:], wst["uhz"], hp,
                                         start=False, stop=True,
                                         skip_group_check=True)

                gbuf = tmp.tile([128, SL], FP16, tag="gt", name=f"gt{k}")
                for t, tl in enumerate(TLS):
                    c0 = t * TLS[0]
                    rt = tmp.tile([128, tl], FP16, tag="rt", name=f"rt{k}_{t}")
                    zt = tmp.tile([128, tl], FP16, tag="zt", name=f"zt{k}_{t}")
                    t1 = tmp.tile([128, tl], FP16, tag="t1", name=f"t1{k}_{t}")
                    at = tmp.tile([128, tl], FP16, tag="at", name=f"at{k}_{t}")
                    gt = gbuf[:, c0:c0 + tl]
                    gate[t] = (rt, zt, t1, at, gt)

                def emit_t1(t):
                    rt, zt, t1, at, gt = gate[t]
                    p_r, p_z, p_a1, p_ia = ps[t]
                    if k == 0:
                        nc.vector.tensor_scalar_mul(t1[:], rt[:],
                                                    t_bn[:, 1:2])
                    else:
                        nc.vector.scalar_tensor_tensor(
                            t1[:], in0=p_a1[:], scalar=t_bn[:, 0:1],
                            in1=rt[:], op0=ALU.add, op1=ALU.mult)
                    nc.tensor.matmul(p_ia[:], wst["ident"], t1[:],
                                     start=False, stop=True,
                                     skip_group_check=True)

                # ACT/DVE in readiness order
                nc.scalar.activation(gate[0][0][:], ps[0][0][:], AF.Sigmoid)
                nc.scalar.activation(gate[0][1][:], ps[0][1][:], AF.Sigmoid)
                emit_t1(0)
                nc.scalar.activation(gate[1][0][:], ps[1][0][:], AF.Sigmoid)
                nc.scalar.activation(f_act[:], gate[0][1][:, 0:170], AF.Sigmoid)
                nc.scalar.activation(gate[0][3][:], ps[0][3][:], AF.Sigmoid)
                emit_t1(1)
                nc.scalar.activation(gate[1][1][:], ps[1][1][:], AF.Sigmoid)

                dmaq = [nc.sync, nc.scalar]
                dmai = [0]

                def emit_dma(blo, bhi):
                    # one 2D DMA moves both segments: yout rows 0:64 = seg0,
                    # rows 64:128 = seg1 (host splits)
                    ylo, yhi = blo - 1 - MARG, bhi - 1 - MARG
                    q0 = dmaq[dmai[0] % 2]
                    dmai[0] += 1
                    q0.dma_start(yout[:, ylo:yhi], hnew[:, blo:bhi])

                rt0, zt0, t10, at0, gt0 = gate[0]
                nc.vector.scalar_tensor_tensor(
                    gt0[:], in0=zt0[:], scalar=1.0, in1=at0[:],
                    op0=ALU.subtract, op1=ALU.mult)
                nc.vector.tensor_copy(f_dve[:], gate[0][3][:, 0:180])
                nc.vector.tensor_tensor_scan(
                    hnew[:, 1:1 + TLS[0]], zt0[:], gt0[:], 0.5,
                    ALU.mult, ALU.subtract)
                if k == ROUNDS - 1:
                    emit_dma(1 + MARG, 1 + TLS[0])

                nc.scalar.activation(gate[1][3][:], ps[1][3][:], AF.Sigmoid)
                rt1, zt1, t11, at1, gt1 = gate[1]
                nc.vector.scalar_tensor_tensor(
                    gt1[:], in0=zt1[:], scalar=1.0, in1=at1[:],
                    op0=ALU.subtract, op1=ALU.mult)
                if k < ROUNDS - 1:
                    nc.vector.tensor_tensor_scan(
                        hnew[:, 1 + TLS[0]:1 + SL], zt1[:], gt1[:],
                        hnew[:, TLS[0]:TLS[0] + 1], ALU.mult, ALU.subtract)
                else:
                    c0 = TLS[0]
                    while c0 < SL:
                        ce = min(c0 + FIN_CHUNK, SL)
                        nc.vector.tensor_tensor_scan(
                            hnew[:, 1 + c0:1 + ce],
                            zt1[:, c0 - TLS[0]:ce - TLS[0]],
                            gt1[:, c0 - TLS[0]:ce - TLS[0]],
                            hnew[:, c0:c0 + 1], ALU.mult, ALU.subtract)
                        emit_dma(1 + c0, 1 + ce)
                        c0 = ce

            for k in range(ROUNDS):
                emit_round(k)

    nc.compile()
    return nc


_CACHE = {}


def kernel(**inputs):
    xs = np.asarray(inputs["xs"], np.float32)
    w_ih = np.asarray(inputs["w_ih"], np.float32)
    w_hh = np.asarray(inputs["w_hh"], np.float32)
    b_gru = np.asarray(inputs["b_gru"], np.float32)
    bn_gru = np.asarray(inputs["bn_gru"], np.float32)

    if "nc" not in _CACHE:
        _CACHE["nc"] = _build_program()
    nc = _CACHE["nc"]

    # H' = (h+1)/2 substitution + tanh->sigmoid doubling, folded into the
    # stationaries:  gpre = Wg x + (bg - Ug@1) + (2 Ug) H'   for g in r,z
    #   2*aarg = (2 Wa) x + 2 ba + r*( (4 Ua) H' + 2(bn - Ua@1) )
    base = np.zeros((128, BLOBCOLS), np.float32)
    gmul = {"wxr": 1.0, "wxz": 1.0, "wxa": 2.0}
    umul = {"uhr": 2.0, "uhz": 2.0, "uha": 4.0}
    for gi, key in enumerate(("wxr", "wxz", "wxa")):
        wg = w_ih[gi * H:(gi + 1) * H]          # (H, NIN)
        ug = w_hh[gi * H:(gi + 1) * H]          # (H, H)
        if key == "wxa":
            bg = 2.0 * b_gru[gi * H:(gi + 1) * H]
            bg0 = bg
        else:
            bg = b_gru[gi * H:(gi + 1) * H] - ug.sum(axis=1)
            bg0 = b_gru[gi * H:(gi + 1) * H]
        for s in range(NSEG):
            r0 = s * (NIN + 1)
            for off, bias in ((WOFF[key], bg), (W0OFF[key], bg0)):
                base[r0:r0 + NIN, off + s * H:off + s * H + H] = \
                    gmul[key] * wg.T
                base[r0 + NIN, off + s * H:off + s * H + H] = bias
    # seg0 mask row: saturate z -> 1 on masked cols (state pass-through)
    base[XR - 1, WOFF["wxz"]:WOFF["wxz"] + H] = ZBIG
    base[XR - 1, W0OFF["wxz"]:W0OFF["wxz"] + H] = ZBIG
    for gi, key in enumerate(("uhr", "uhz", "uha")):
        ug = w_hh[gi * H:(gi + 1) * H]
        for s in range(NSEG):
            base[s * H:(s + 1) * H,
                 UOFF[key] + s * H:UOFF[key] + (s + 1) * H] = umul[key] * ug.T
    base[:, IDOFF:IDOFF + 128] = np.eye(128, dtype=np.float32)
    bnpp = 2.0 * (bn_gru - w_hh[2 * H:].sum(axis=1))
    base[0:H, BNOFF] = bnpp
    base[0:H, BNOFF + 1] = 2.0 * bn_gru

    in_maps = []
    for core in range(N_CORES):
        bi, half = core // 2, core % 2
        p0 = half * TPC - MARG
        xw = np.zeros((NW, NIN), np.float32)
        lo = max(0, p0)
        xw[lo - p0:] = xs[bi, lo:p0 + NW]
        blob = base.copy()
        for s in range(NSEG):
            r0 = s * (NIN + 1)
            xsg = xw[512 * s:512 * s + SL]       # (SL, NIN)
            blob[r0:r0 + NIN, XOFF:XOFF + SL] = xsg.T
            blob[r0 + NIN, XOFF:XOFF + SL] = 1.0
        if half == 0:
            blob[XR - 1, XOFF:XOFF + MARG] = 1.0
        in_maps.append({"wx": blob.astype(ml_dtypes.bfloat16)})

    results = run_bass_kernel_spmd(nc, in_maps, list(range(N_CORES))).results

    out = np.empty((B, L, H), np.float32)
    for core in range(N_CORES):
        bi, half = core // 2, core % 2
        y = np.asarray(results[core]["y"]).astype(np.float32)   # (128, 512)
        yh = np.concatenate([y[0:H], y[H:128]], axis=1)          # (64, 1024)
        out[bi, half * TPC:(half + 1) * TPC] = 2.0 * yh.T - 1.0
    return out


# revision 17
# speedup vs baseline: 1.0037x; 1.0023x over previous
"""Trainium2 Bass kernel for nn_CellLayer_25752623907073.

The reference is an init-guess network (MLP/S4D stack) followed by a DEER
quasi-Newton parallel solve of a GRU recurrence.  DEER is a contraction: it
converges to the sequential GRU trajectory from any initial guess, so the
init-guess network has no effect on the output and the task reduces to
evaluating the GRU trajectory.

The kernel solves the GRU by quasi-DEER fixed-point iteration with a
DIAGONAL linear solve: each round evaluates all gates in parallel at the
lagged previous iterate h~[t-1], then propagates the exact diagonal
recurrence h[t] = z[t]*h[t-1] + (1-z[t])*a[t] along the sequence with DVE
tensor_tensor_scan ops (state fp32 inside the scan).  The fixed point is
the true trajectory; measured contraction ~0.26/round, so ROUNDS=4 puts
iteration error under the bf16 floor (total rel err ~8e-3 vs 2e-2 gate).

Implementation notes:
- State is carried as H' = (h+1)/2, which turns the a-gate tanh into a
  sigmoid (tanh(x) = 2*sigmoid(2x)-1) with all affine corrections folded
  into host-side weight/bias transforms:
      H'[t] = z[t]*H'[t-1] - (z[t]-1)*s[t],   s = sigmoid(2*aarg).
  Only the sigmoid ACT table is ever needed (one 1283ns table load, warmed
  during the input DMA), and no extra DVE op for the *2-1 affine.
- Sharding: 8 cores = 4 batches x 2 halves, no collectives.  Each core owns
  MARG+1024 positions; the window splits into two 528-col segments stacked
  on partitions (seg0 -> 0:64, seg1 -> 64:128) so every ACT/DVE op covers
  2x columns; matmuls use block-diagonal stationaries.  Segment scan
  initials are H'=0.5 (h=0) every round; the injected boundary error
  decays below 1e-3 within the MARG warm-in cols, which are discarded.
- The first-half core's seg0 warm-in cols have no real inputs: a dedicated
  x2 mask row drives the z pre-activation to +32 there, so z=1 and the
  scan passes the initial state through exactly (no mask op needed).
- Per 264-col tile and round: 7 bf16 matmuls (x-parts emitted early /
  h-parts close each psum so sigmoids fire right after the scan lands),
  sigmoids for r/z/a, t1=(ha+bn'')*r on DVE, a PE identity-matmul folding
  t1 onto the ia psum bank, then g=(z-1)*s and the scan back-to-back on
  DVE.  Per-engine emission follows steady-state readiness: the
  completion-counter semaphores entangle waiters when instructions park.
- Round 0 runs the generic code against an H'=0.5 buffer (h=0 exactly).
- y is DMA'd as bf16 H' (the last round's tile-1 scan is chunked so the
  DMAs pipeline on the SP/ACT HWDGE queues); the host maps 2*H'-1.
"""

import numpy as np
import ml_dtypes

import concourse.bacc as bacc
import concourse.mybir as mybir
import concourse.tile as tile
from concourse.bass_utils import run_bass_kernel_spmd

F32 = mybir.dt.float32
BF16 = mybir.dt.bfloat16
FP16 = mybir.dt.float16
AF = mybir.ActivationFunctionType
ALU = mybir.AluOpType

B, L, NIN, H = 4, 2048, 32, 64
TPC = L // 2          # timesteps per core
MARG = 16             # warm-in columns per segment (discarded)
SL = MARG + 512       # segment length (528)
NW = MARG + TPC       # window length (1040)
NSEG = 2
TLS = [264, 264]      # tile column sizes (sum = SL)
FIN_CHUNK = 264       # last-round scan/DMA chunk size
ROUNDS = 4
N_CORES = 8
ZBIG = 32.0           # z pre-act saturation for masked warm-in cols
XR = 2 * (NIN + 1) + 1  # x2 rows: 2 segs x (x + ones) + seg0 mask row

# blob columns (bf16, [128, BLOBCOLS]) — DMA'd in 4 pieces:
#   A (SP):  [0:513]  wxr|wxz|wxa block-diag stationaries (x rows, bias row,
#            mask row), I128, bn'' col
#   B (DVE): x2 tile-0 cols
#   C (SP):  x2 tile-1 cols
#   D (SP):  uhr|uhz|uha block-diag stationaries (needed from round 0)
W0OFF = {"wxr": 0, "wxz": 128, "wxa": 256}     # round-0 bias variant
BNOFF = 384            # col 384 = bn'' (rounds>0), col 385 = 2*bn (round 0)
XOFF = 386             # DMA-A covers partitions 0:XR, cols [0:XOFF+SL)
IDOFF = XOFF + SL      # DMA-B: ident, generic wx, uh stationaries
WOFF = {"wxr": IDOFF + 128, "wxz": IDOFF + 256, "wxa": IDOFF + 384}
UOFF = {"uhr": IDOFF + 512, "uhz": IDOFF + 640, "uha": IDOFF + 768}
BLOBCOLS = IDOFF + 896


def _build_program():
    nc = bacc.Bacc("TRN2", debug=False)

    wx = nc.declare_dram_parameter("wx", [128, BLOBCOLS], BF16, isOutput=False)
    yout = nc.declare_dram_parameter("y", [128, 512], FP16, isOutput=True)

    with tile.TileContext(nc) as tc:
        with (
            tc.tile_pool(name="const", bufs=1) as cpool,
            tc.tile_pool(name="tmp", bufs=2) as tmp,
            tc.tile_pool(name="psum_r", bufs=2, space="PSUM") as psum_r,
            tc.tile_pool(name="psum_z", bufs=2, space="PSUM") as psum_z,
            tc.tile_pool(name="psum_a1", bufs=2, space="PSUM") as psum_a1,
            tc.tile_pool(name="psum_ia", bufs=2, space="PSUM") as psum_ia,
        ):
            t_wx = cpool.tile([128, BLOBCOLS], BF16)
            nc.sync.dma_start(t_wx[0:XR, 0:XOFF + TLS[0]],
                              wx[0:XR, 0:XOFF + TLS[0]])
            nc.sync.dma_start(t_wx[:, XOFF + TLS[0]:IDOFF + 128],
                              wx[:, XOFF + TLS[0]:IDOFF + 128])
            nc.sync.dma_start(t_wx[:, IDOFF + 128:BLOBCOLS],
                              wx[:, IDOFF + 128:BLOBCOLS])

            wst = {k: t_wx[:, off:off + 128] for k, off in UOFF.items()}
            wst["ident"] = t_wx[:, IDOFF:IDOFF + 128]
            wx_x = {k: t_wx[0:XR, WOFF[k]:WOFF[k] + 128]
                    for k in ("wxr", "wxz", "wxa")}
            wx_x0 = {k: t_wx[0:XR, W0OFF[k]:W0OFF[k] + 128]
                     for k in ("wxr", "wxz", "wxa")}

            # PE p-state warm-up: dummy matmuls on a zeroed tile so the ramp
            # to full clock runs during the input DMA.
            t_zero = cpool.tile([128, 264], BF16)
            nc.vector.memset(t_zero[:], 0.0)
            for i in range(2):
                p_w = psum_ia.tile([128, 264], F32, tag="pia", name=f"warm{i}")
                nc.tensor.matmul(p_w[:], t_zero[:, 0:128], t_zero[:],
                                 start=True, stop=True)

            # warm the sigmoid ACT table during the input DMA
            t_warm = cpool.tile([1, 1], F32)
            nc.vector.memset(t_warm[:], 0.0)
            nc.scalar.activation(t_warm[:], t_warm[:], AF.Sigmoid)

            # greedy-scheduler engine reservations: small always-ready ops
            # emitted just before a critical op whose deps are in ack-flight,
            # so the list scheduler doesn't give the slot to a long off-path
            # op instead.
            f_dve = cpool.tile([128, 180], FP16)
            f_act = cpool.tile([128, 200], F32)

            # bn'' / 2bn as f32 via DVE so t1 never waits on the DMA sem;
            # the blob carries rows 0:64 only (partition-packed first DMA),
            # ACT replicates into partitions 64:128.
            t_bn = cpool.tile([128, 2], F32)
            nc.vector.tensor_copy(t_bn[0:H, :], t_wx[0:H, BNOFF:BNOFF + 2])
            nc.scalar.activation(t_bn[H:128, :], t_bn[0:H, :], AF.Copy)

            # H' double buffers: col 0 = 0.5 (h=0 before the segment), col
            # 1+c = state at segment-local col c.  hbuf[0] is fully 0.5 so
            # round 0 is the generic code evaluated at h=0.
            hbuf = [cpool.tile([128, 1 + SL], FP16, name=f"h{i}")
                    for i in range(2)]
            nc.vector.memset(hbuf[0][:, 0:1], 0.5)
            nc.vector.memset(hbuf[1][:, 0:1], 0.5)

            def x2(c0, tl):
                return t_wx[0:XR, XOFF + c0:XOFF + c0 + tl]

            def emit_round(k):
                hprev = hbuf[k % 2]
                hnew = hbuf[(k + 1) % 2]
                ps = {}
                gate = {}
                # PE: x-part starts (ready as soon as the psum bank WAR
                # clears, run in earlier idle slots), then h-part closes in
                # readiness order, then the ident folds.
                wxk = wx_x0 if k == 0 else wx_x
                for t, tl in enumerate(TLS):
                    c0 = t * TLS[0]
                    xa = x2(c0, tl)
                    p_r = psum_r.tile([128, tl], F32, tag="pr",
                                      name=f"pr_{k}_{t}")
                    p_z = psum_z.tile([128, tl], F32, tag="pz",
                                      name=f"pz_{k}_{t}")
                    p_a1 = psum_a1.tile([128, tl], F32, tag="pa1",
                                        name=f"pa1_{k}_{t}")
                    p_ia = psum_ia.tile([128, tl], F32, tag="pia",
                                        name=f"pia_{k}_{t}")
                    last = k == 0
                    nc.tensor.matmul(p_r[:], wxk["wxr"], xa,
                                     start=True, stop=last)
                    nc.tensor.matmul(p_z[:], wxk["wxz"], xa,
                                     start=True, stop=last,
                                     skip_group_check=True)
                    nc.tensor.matmul(p_ia[:], wxk["wxa"], xa,
                                     start=True, stop=False,
                                     skip_group_check=True)
                    ps[t] = (p_r, p_z, p_a1, p_ia)
                if k > 0:
                    for t, tl in enumerate(TLS):
                        c0 = t * TLS[0]
                        hp = hprev[:, c0:c0 + tl]
                        p_r, p_z, p_a1, p_ia = ps[t]
                        nc.tensor.matmul(p_r[:], wst["uhr"], hp,
                                         start=False, stop=True,
                                         skip_group_check=True)
                        nc.tensor.matmul(p_a1[:], wst["uha"], hp,
                                         start=True, stop=True,
                                         skip_group_check=True)
                        nc.tensor.matmul(p_z[:], wst["uhz"], hp,
                                         start=False, stop=True,
                                         skip_group_check=True)

                gbuf = tmp.tile([128, SL], FP16, tag="gt", name=f"gt{k}")
                for t, tl in enumerate(TLS):
                    c0 = t * TLS[0]
                    rt = tmp.tile([128, tl], FP16, tag="rt", name=f"rt{k}_{t}")
                    zt = tmp.tile([128, tl], FP16, tag="zt", name=f"zt{k}_{t}")
                    t1 = tmp.tile([128, tl], FP16, tag="t1", name=f"t1{k}_{t}")
                    at = tmp.tile([128, tl], FP16, tag="at", name=f"at{k}_{t}")
                    gt = gbuf[:, c0:c0 + tl]
                    gate[t] = (rt, zt, t1, at, gt)

                def emit_t1(t):
                    rt, zt, t1, at, gt = gate[t]
                    p_r, p_z, p_a1, p_ia = ps[t]
                    if k == 0:
                        nc.vector.tensor_scalar_mul(t1[:], rt[:],
                                                    t_bn[:, 1:2])
                    else:
                        nc.vector.scalar_tensor_tensor(
                            t1[:], in0=p_a1[:], scalar=t_bn[:, 0:1],
                            in1=rt[:], op0=ALU.add, op1=ALU.mult)
                    nc.tensor.matmul(p_ia[:], wst["ident"], t1[:],
                                     start=False, stop=True,
                                     skip_group_check=True)

                # ACT/DVE in readiness order
                nc.scalar.activation(gate[0][0][:], ps[0][0][:], AF.Sigmoid)
                nc.scalar.activation(gate[0][1][:], ps[0][1][:], AF.Sigmoid)
                emit_t1(0)
                nc.scalar.activation(gate[1][0][:], ps[1][0][:], AF.Sigmoid)
                nc.scalar.activation(f_act[:], gate[0][1][:, 0:200], AF.Sigmoid)
                nc.scalar.activation(gate[0][3][:], ps[0][3][:], AF.Sigmoid)
                emit_t1(1)
                nc.scalar.activation(gate[1][1][:], ps[1][1][:], AF.Sigmoid)

                dmaq = [nc.sync, nc.scalar]
                dmai = [0]

                def emit_dma(blo, bhi):
                    # one 2D DMA moves both segments: yout rows 0:64 = seg0,
                    # rows 64:128 = seg1 (host splits)
                    ylo, yhi = blo - 1 - MARG, bhi - 1 - MARG
                    q0 = dmaq[dmai[0] % 2]
                    dmai[0] += 1
                    q0.dma_start(yout[:, ylo:yhi], hnew[:, blo:bhi])

                rt0, zt0, t10, at0, gt0 = gate[0]
                nc.vector.scalar_tensor_tensor(
                    gt0[:], in0=zt0[:], scalar=1.0, in1=at0[:],
                    op0=ALU.subtract, op1=ALU.mult)
                nc.vector.tensor_copy(f_dve[:], gate[0][3][:, 0:180])
                nc.vector.tensor_tensor_scan(
                    hnew[:, 1:1 + TLS[0]], zt0[:], gt0[:], 0.5,
                    ALU.mult, ALU.subtract)
                if k == ROUNDS - 1:
                    emit_dma(1 + MARG, 1 + TLS[0])

                nc.scalar.activation(gate[1][3][:], ps[1][3][:], AF.Sigmoid)
                rt1, zt1, t11, at1, gt1 = gate[1]
                nc.vector.scalar_tensor_tensor(
                    gt1[:], in0=zt1[:], scalar=1.0, in1=at1[:],
                    op0=ALU.subtract, op1=ALU.mult)
                if k < ROUNDS - 1:
                    nc.vector.tensor_tensor_scan(
                        hnew[:, 1 + TLS[0]:1 + SL], zt1[:], gt1[:],
                        hnew[:, TLS[0]:TLS[0] + 1], ALU.mult, ALU.subtract)
                else:
                    c0 = TLS[0]
                    while c0 < SL:
                        ce = min(c0 + FIN_CHUNK, SL)
                        nc.vector.tensor_tensor_scan(
                            hnew[:, 1 + c0:1 + ce],
                            zt1[:, c0 - TLS[0]:ce - TLS[0]],
                            gt1[:, c0 - TLS[0]:ce - TLS[0]],
                            hnew[:, c0:c0 + 1], ALU.mult, ALU.subtract)
                        emit_dma(1 + c0, 1 + ce)
                        c0 = ce

            for k in range(ROUNDS):
                emit_round(k)

    nc.compile()
    return nc


_CACHE = {}


def kernel(**inputs):
    xs = np.asarray(inputs["xs"], np.float32)
    w_ih = np.asarray(inputs["w_ih"], np.float32)
    w_hh = np.asarray(inputs["w_hh"], np.float32)
    b_gru = np.asarray(inputs["b_gru"], np.float32)
    bn_gru = np.asarray(inputs["bn_gru"], np.float32)

    if "nc" not in _CACHE:
        _CACHE["nc"] = _build_program()
    nc = _CACHE["nc"]

    # H' = (h+1)/2 substitution + tanh->sigmoid doubling, folded into the
    # stationaries:  gpre = Wg x + (bg - Ug@1) + (2 Ug) H'   for g in r,z
    #   2*aarg = (2 Wa) x + 2 ba + r*( (4 Ua) H' + 2(bn - Ua@1) )
    base = np.zeros((128, BLOBCOLS), np.float32)
    gmul = {"wxr": 1.0, "wxz": 1.0, "wxa": 2.0}
    umul = {"uhr": 2.0, "uhz": 2.0, "uha": 4.0}
    for gi, key in enumerate(("wxr", "wxz", "wxa")):
        wg = w_ih[gi * H:(gi + 1) * H]          # (H, NIN)
        ug = w_hh[gi * H:(gi + 1) * H]          # (H, H)
        if key == "wxa":
            bg = 2.0 * b_gru[gi * H:(gi + 1) * H]
            bg0 = bg
        else:
            bg = b_gru[gi * H:(gi + 1) * H] - ug.sum(axis=1)
            bg0 = b_gru[gi * H:(gi + 1) * H]
        for s in range(NSEG):
            r0 = s * (NIN + 1)
            for off, bias in ((WOFF[key], bg), (W0OFF[key], bg0)):
                base[r0:r0 + NIN, off + s * H:off + s * H + H] = \
                    gmul[key] * wg.T
                base[r0 + NIN, off + s * H:off + s * H + H] = bias
    # seg0 mask row: saturate z -> 1 on masked cols (state pass-through)
    base[XR - 1, WOFF["wxz"]:WOFF["wxz"] + H] = ZBIG
    base[XR - 1, W0OFF["wxz"]:W0OFF["wxz"] + H] = ZBIG
    for gi, key in enumerate(("uhr", "uhz", "uha")):
        ug = w_hh[gi * H:(gi + 1) * H]
        for s in range(NSEG):
            base[s * H:(s + 1) * H,
                 UOFF[key] + s * H:UOFF[key] + (s + 1) * H] = umul[key] * ug.T
    base[:, IDOFF:IDOFF + 128] = np.eye(128, dtype=np.float32)
    bnpp = 2.0 * (bn_gru - w_hh[2 * H:].sum(axis=1))
    base[0:H, BNOFF] = bnpp
    base[0:H, BNOFF + 1] = 2.0 * bn_gru

    in_maps = []
    for core in range(N_CORES):
        bi, half = core // 2, core % 2
        p0 = half * TPC - MARG
        xw = np.zeros((NW, NIN), np.float32)
        lo = max(0, p0)
        xw[lo - p0:] = xs[bi, lo:p0 + NW]
        blob = base.copy()
        for s in range(NSEG):
            r0 = s * (NIN + 1)
            xsg = xw[512 * s:512 * s + SL]       # (SL, NIN)
            blob[r0:r0 + NIN, XOFF:XOFF + SL] = xsg.T
            blob[r0 + NIN, XOFF:XOFF + SL] = 1.0
        if half == 0:
            blob[XR - 1, XOFF:XOFF + MARG] = 1.0
        in_maps.append({"wx": blob.astype(ml_dtypes.bfloat16)})

    results = run_bass_kernel_spmd(nc, in_maps, list(range(N_CORES))).results

    out = np.empty((B, L, H), np.float32)
    for core in range(N_CORES):
        bi, half = core // 2, core % 2
        y = np.asarray(results[core]["y"]).astype(np.float32)   # (128, 512)
        yh = np.concatenate([y[0:H], y[H:128]], axis=1)          # (64, 1024)
        out[bi, half * TPC:(half + 1) * TPC] = 2.0 * yh.T - 1.0
    return out
